# revision 25
# baseline (speedup 1.0000x reference)
"""Trainium2 Bass kernel for nn_Expert_Gate (MMoE: 8 experts, 2 task gates).

Reference computation (all dense, fp32):
    h      = relu(einsum('bi,eih->ebh', x, W1) + b1)          [E, B, H1]
    e_out  = relu(einsum('ebh,eho->ebo', h, W2) + b2)         [E, B, H2]
    gates  = softmax(einsum('bi,tie->tbe', x, Wg) + bg, -1)   [T, B, E]
    towers = einsum('tbe,ebo->tbo', gates, e_out)             [T, B, H2]

Sharding: pure data-parallel over batch. Each of the 8 cores gets B/8 = 2048
rows of x, all weights replicated, no collectives.

Per-core dataflow (Bc = 2048, processed in 4 chunks of 512 rows, each chunk
as 4 b-tiles of 128):
  - x is host-transposed to xT [I, Bc] and converted fp16 (as are W1/W2/Wg):
    PE runs 1 cycle/row, DMA traffic and SBUF halve vs fp32.
  - L1: h.T chunks [h1, b] = W1[e].T @ xT  (N=512) -> PSUM, relu+bias copied
    to SBUF as fp16 in [h1, b] layout on ACT/DVE/Pool round-robin.
  - gate logits computed transposed (Wg stationary, N=512), exp+bias fused
    into the PSUM->SBUF copy on ACT, PE-transposed back to [128b, 16];
    softmax normalization on DVE -> gate weights w [128b, (e,t)] fp16.
  - L2: e_out chunks [b, o] = hT_slice.T @ W2[e]  (fp16, N=128) -> stacked
    PSUM [128b, (8e,128o)], relu to SBUF fp16.
  - combine on the PE via diagonal matmuls: towers.T[o, (t,b)] accumulates
    sum_e (eout_e).T @ diag(w_te) in PSUM.  diag tiles are built once per
    b-tile on DVE/GpSimd as ident*w with broadcast APs.
  - towers copied to SBUF fp16 and DMA'd out as [T, H2, Bc]; host transposes
    back and upcasts.

Timing mode (reps>1): the body (loads + compute + store = one full
inference) is emitted `unroll` times inside a tc.For_i hardware loop —
the loop's all-engine barrier (~6us) amortizes over `unroll` bodies.
Input tiles come from bufs=2 pools so body u+1's DMAs overlap body u's
compute.
"""

import sys
from contextlib import ExitStack

import numpy as np

if "/opt/trn_rl_repo" not in sys.path:
    sys.path.append("/opt/trn_rl_repo")

import concourse.bass as bass  # noqa: E402
import concourse.tile as tile  # noqa: E402
from concourse import bacc, mybir  # noqa: E402
from concourse.bass_utils import run_bass_kernel_spmd  # noqa: E402

F32 = mybir.dt.float32
F16 = mybir.dt.float16
AF = mybir.ActivationFunctionType
ALU = mybir.AluOpType

B, I, H1, H2, E, T = 16384, 512, 256, 128, 8, 2
NCORES = 8
BC = B // NCORES          # 2048 rows per core
CHUNK = 512               # rows per pipeline chunk (PSUM free-dim limit)
NCHUNK = BC // CHUNK      # 4
NBT = CHUNK // 128        # 4 b-tiles per chunk
KC_I = I // 128           # 4 contraction chunks for layer 1 / gates
MC_H1 = H1 // 128         # 2 output chunks for layer 1 == K chunks for layer 2
UNROLL = 4                # inference bodies per hardware-loop iteration

_CACHE: dict = {}

EO_ALL_ACT = False
# "elem": gate-weighted combine as elementwise mult+reduce on Pool/DVE.
#         Sim-validated (rel err 8.9e-4, PE busy 72us, DVE 68.5us, total
#         90.5us single-shot) but NOT hardware-verified — do not enable
#         without a full test.py run.
# "diag": combine on the PE via diagonal matmuls (hardware-verified path,
#         101861 ns / inference).
COMBINE = "diag"


def _emit(nc, t, has_b2: bool, reps: int = 1, parts: str = "full",
          loop_loads: bool = False, unroll: int = UNROLL):
    """Emit the per-core program. `t` maps tensor names -> DRAM APs.

    reps>1 wraps `unroll` complete inference bodies (input DMAs + compute
    + store) in a hardware For_i loop for timing; one loop iteration ==
    `unroll` full inferences.
    """
    del loop_loads  # loads are always per body now
    P = {"gates", "diag", "l1", "l2", "combine"} if parts == "full" else set(
        parts.split(",")
    )
    if reps == 1:
        unroll = 1
    with tile.TileContext(nc) as tc, ExitStack() as ctx:
        const = ctx.enter_context(tc.tile_pool(name="const", bufs=1))
        xt_p = ctx.enter_context(tc.tile_pool(name="xtp", bufs=2))
        w1_p = ctx.enter_context(tc.tile_pool(name="w1p", bufs=2))
        w2_p = ctx.enter_context(tc.tile_pool(name="w2p", bufs=2))
        co_p = ctx.enter_context(tc.tile_pool(name="cop", bufs=2))
        wg_p = ctx.enter_context(tc.tile_pool(name="wgp", bufs=2))
        ht_p = ctx.enter_context(tc.tile_pool(name="ht", bufs=14))
        eo_p = ctx.enter_context(tc.tile_pool(name="eo", bufs=5))
        sm_p = ctx.enter_context(tc.tile_pool(name="sm", bufs=3))
        if COMBINE == "elem":
            pr_p = ctx.enter_context(tc.tile_pool(name="pr", bufs=3))
            tw_p = ctx.enter_context(tc.tile_pool(name="tw", bufs=2))
            hps_p = ctx.enter_context(
                tc.tile_pool(name="hps", bufs=4, space="PSUM")
            )
            eps_p = ctx.enter_context(
                tc.tile_pool(name="eps", bufs=3, space="PSUM")
            )
        else:
            dg_p = ctx.enter_context(tc.tile_pool(name="dg", bufs=7))
            hps_p = ctx.enter_context(
                tc.tile_pool(name="hps", bufs=3, space="PSUM")
            )
            eps_p = ctx.enter_context(
                tc.tile_pool(name="eps", bufs=2, space="PSUM")
            )
            tps_p = ctx.enter_context(
                tc.tile_pool(name="tps", bufs=2, space="PSUM")
            )
        gps_p = ctx.enter_context(tc.tile_pool(name="gps", bufs=1, space="PSUM"))

        if COMBINE == "diag":
            outT = const.tile([128, T, BC], F16)        # towers.T staging
        if has_b2:
            b2_sb = const.tile([1, E * H2], F16)
            on_sb = const.tile([1, 128], F16)
            nc.sync.dma_start(out=b2_sb[:], in_=t["b2r"])
            nc.sync.dma_start(out=on_sb[:], in_=t["ones1"])

        xt_r = t["xt"].rearrange("(kc p) b -> p kc b", p=128)
        w1_r = t["w1"].rearrange("e (kc p) m -> p e kc m", p=128)

        # dummies for part-disabled timing builds
        if "gates" not in P:
            wn_dummy = const.tile([128, NBT * E * T], F16)
            nc.vector.memset(wn_dummy[:], 0.125)
        if "diag" not in P and COMBINE == "diag":
            dg_dummy = const.tile([128, E * T, 128], F16)
            nc.vector.memset(dg_dummy[:], 0.125)
        if "l1" not in P:
            ht_dummy = const.tile([128, MC_H1, CHUNK], F16)
            nc.vector.memset(ht_dummy[:], 0.125)
        if "l2" not in P:
            eo_dummy = const.tile([128, E, H2], F16)
            nc.vector.memset(eo_dummy[:], 0.125)
        if "combine" not in P:
            if COMBINE == "diag":
                nc.vector.memset(outT[:], 0.0)
            else:
                tw_dummy = const.tile([128, NBT, T, H2], F16)
                nc.vector.memset(
                    tw_dummy[:].rearrange("p a t o -> p (a t o)"), 0.0
                )

        def load_body():
            """Allocate this body's input tiles and start their DMAs.

            Double-buffered pools: the WAR dependency of each DMA is
            against the body-before-last, so all transfers start while
            the previous body computes.  First-needed data (x chunk 0
            kc0, consts, wg, W1 e=0) lands first.  HWDGE triggers cost
            ~625ns serial on the issuing queue, so the DMA count stays
            low: x/consts/wg on the SP queue, W1 on the Pool queue, W2
            on the ACT queue (ahead of the out DMAs).
            """
            xt_sb = xt_p.tile([128, KC_I, BC], F16)
            w1_sb = w1_p.tile([128, E, KC_I, H1], F16)
            w2_sb = w2_p.tile([128, E, MC_H1, H2], F16)
            wg_sb = wg_p.tile([128, KC_I, E * T], F16)
            # packed small constants:
            # [b1 (16) | ident-f16-as-f32 (64) | id32 (16) | bg (1)]
            co_sb = co_p.tile([128, 97], F32)
            nc.sync.dma_start(out=xt_sb[:, 0, 0:CHUNK], in_=xt_r[:, 0, 0:CHUNK])
            nc.gpsimd.dma_start(out=w1_sb[:, 0], in_=w1_r[:, 0])
            nc.sync.dma_start(out=co_sb[:], in_=t["consts"])
            nc.sync.dma_start(
                out=wg_sb[:], in_=t["wg"].rearrange("(kc p) g -> p kc g", p=128)
            )
            nc.sync.dma_start(
                out=xt_sb[:, 1:, 0:CHUNK], in_=xt_r[:, 1:, 0:CHUNK]
            )
            nc.gpsimd.dma_start(out=w1_sb[:, 1], in_=w1_r[:, 1])
            nc.gpsimd.dma_start(out=w1_sb[:, 2:4], in_=w1_r[:, 2:4])
            nc.scalar.dma_start(
                out=w2_sb[:],
                in_=t["w2"].rearrange("e (kc p) o -> p e kc o", p=128),
            )
            nc.gpsimd.dma_start(out=w1_sb[:, 4:8], in_=w1_r[:, 4:8])
            for c in (1, 2, 3):
                sl = slice(c * CHUNK, (c + 1) * CHUNK)
                nc.sync.dma_start(out=xt_sb[:, :, sl], in_=xt_r[:, :, sl])
            return xt_sb, w1_sb, w2_sb, wg_sb, co_sb

        def body():
            xt_sb, w1_sb, w2_sb, wg_sb, co_sb = load_body()
            b1_sb = co_sb[:, 0:16]
            id_sb = co_sb[:, 16:80].bitcast(F16)
            id32_sb = co_sb[:16, 80:96]
            bg_sb = co_sb[:16, 96:97]

            def gates_and_diag(c):
                """Gate softmax weights + diag tiles for chunk c."""
                cs = c * CHUNK
                if "gates" in P:
                    lt_ps = gps_p.tile([16, CHUNK], F32, tag="g")
                    for kc in range(KC_I):
                        nc.tensor.matmul(
                            lt_ps[:],
                            wg_sb[:, kc, :],
                            xt_sb[:, kc, cs : cs + CHUNK],
                            start=(kc == 0),
                            stop=(kc == KC_I - 1),
                        )
                    # exp(logits + bg) while leaving PSUM, then transpose back
                    ew = sm_p.tile([16, CHUNK], F32)
                    nc.scalar.activation(
                        ew[:], lt_ps[:], AF.Exp, bias=bg_sb[:, 0:1]
                    )
                    gps = gps_p.tile([128, NBT, E * T], F32, tag="g")
                    for bt in range(NBT):
                        nc.tensor.transpose(
                            gps[:, bt, :],
                            ew[:, bt * 128 : (bt + 1) * 128],
                            id32_sb[:],
                        )
                    # sum over e (col index = bt*16 + e*2 + t)
                    sums = sm_p.tile([128, NBT * T], F32)
                    nc.vector.reduce_sum(
                        sums[:].rearrange("p (bt t) -> p bt t", t=T),
                        gps[:].rearrange("p bt (e t) -> p bt t e", e=E, t=T),
                        axis=mybir.AxisListType.X,
                    )
                    recip = sm_p.tile([128, NBT * T], F32)
                    nc.vector.reciprocal(recip[:], sums[:])
                    wn = sm_p.tile([128, NBT * E * T], F16)
                    nc.vector.tensor_mul(
                        wn[:].rearrange("p (bt e t) -> p bt e t", e=E, t=T),
                        gps[:].rearrange("p bt (e t) -> p bt e t", e=E, t=T),
                        recip[:]
                        .rearrange("p (bt t) -> p bt t", t=T)
                        .unsqueeze(2)
                        .broadcast_to([128, NBT, E, T]),
                    )
                else:
                    wn = wn_dummy
                if COMBINE == "elem":
                    return wn
                if "diag" in P:
                    dgs = []
                    for bt in range(NBT):
                        dg = dg_p.tile([128, E * T, 128], F16)
                        # Pool is SBUF-only (no PSUM access); give it all
                        # diag builds, keeping ACT/DVE for the PSUM copies
                        eng = nc.gpsimd if bt != 0 else nc.vector
                        eng.tensor_mul(
                            dg[:],
                            id_sb[:]
                            .unsqueeze(1)
                            .broadcast_to([128, E * T, 128]),
                            wn[:, bt * E * T : (bt + 1) * E * T]
                            .unsqueeze(2)
                            .broadcast_to([128, E * T, 128]),
                        )
                        dgs.append(dg)
                else:
                    dgs = [dg_dummy] * NBT
                return dgs

            pending: dict = {}
            for c in range(NCHUNK):
                cs = c * CHUNK

                # ---- layer 1 + relu (per expert); chunk c+1's gates+diag
                # are emitted mid-phase so their PE/ACT/DVE ops hide under
                # L1 ----
                hts = []
                for e in range(E):
                    if "l1" in P:
                        ht = ht_p.tile([128, MC_H1, CHUNK], F16)
                        for mc in range(MC_H1):
                            hp = hps_p.tile([128, CHUNK], F32)
                            for kc in range(KC_I):
                                nc.tensor.matmul(
                                    hp[:],
                                    w1_sb[:, e, kc, mc * 128 : (mc + 1) * 128],
                                    xt_sb[:, kc, cs : cs + CHUNK],
                                    start=(kc == 0),
                                    stop=(kc == KC_I - 1),
                                )
                            idx = e * MC_H1 + mc
                            bcol = b1_sb[:, idx : idx + 1]
                            # PSUM readers must be ACT or DVE (Pool is
                            # SBUF-only on hardware)
                            if idx % 2 == 0:
                                nc.scalar.activation(
                                    ht[:, mc, :], hp[:], AF.Relu, bias=bcol
                                )
                            else:
                                nc.vector.tensor_scalar(
                                    ht[:, mc, :], hp[:], bcol, 0.0,
                                    op0=ALU.add, op1=ALU.max,
                                )
                        hts.append(ht)
                    else:
                        hts.append(ht_dummy)
                    if c == 0 and e == 0:
                        pending[0] = gates_and_diag(0)
                    if e == 3 and c + 1 < NCHUNK:
                        pending[c + 1] = gates_and_diag(c + 1)
                dgs = pending.pop(c)

                # ---- layer 2 + relu + combine, software-pipelined per
                # b-tile: combine(bt-1) is emitted after L2(bt) so the
                # relu-copy latency hides under the next tile's L2 ----
                def emit_l2(bt):
                    bsl = slice(bt * 128, (bt + 1) * 128)
                    eo = eo_p.tile([128, E, H2], F16)
                    for half in range(2):
                        eps = eps_p.tile([128, E // 2, H2], F32)
                        for ei in range(E // 2):
                            e = half * (E // 2) + ei
                            for kc in range(MC_H1):
                                nc.tensor.matmul(
                                    eps[:, ei, :],
                                    hts[e][:, kc, bsl],
                                    w2_sb[:, e, kc, :],
                                    start=(kc == 0),
                                    stop=(kc == MC_H1 - 1 and not has_b2),
                                )
                            if has_b2:
                                nc.tensor.matmul(
                                    eps[:, ei, :],
                                    on_sb[:1, :],
                                    b2_sb[:1, e * H2 : (e + 1) * H2],
                                    start=False,
                                    stop=True,
                                )
                        eo_sl = eo[
                            :, half * (E // 2) : (half + 1) * (E // 2), :
                        ]
                        eidx = bt * 2 + half
                        if eidx % 2 == 0 or EO_ALL_ACT:
                            nc.scalar.activation(
                                eo_sl.rearrange("p e o -> p (e o)"),
                                eps[:].rearrange("p e o -> p (e o)"),
                                AF.Relu,
                            )
                        else:
                            nc.vector.tensor_scalar_max(
                                eo_sl.rearrange("p e o -> p (e o)"),
                                eps[:].rearrange("p e o -> p (e o)"),
                                0.0,
                            )
                    return eo

                def emit_combine(bt, eo):
                    tps = tps_p.tile([128, T, 128], F32)
                    tflat = tps[:].rearrange("p t b -> p (t b)")
                    for e in range(E):
                        nc.tensor.matmul(
                            tflat,
                            eo[:, e, :],
                            dgs[bt][:, e * T : (e + 1) * T, :].rearrange(
                                "p t b -> p (t b)"
                            ),
                            start=(e == 0),
                            stop=(e == E - 1),
                        )
                    dst = outT[:, :, cs + bt * 128 : cs + (bt + 1) * 128]
                    if bt % 2 == 0:
                        nc.vector.tensor_copy(dst, tps[:])
                    else:
                        nc.scalar.copy(dst, tps[:])
                    if bt % 2 == 1:
                        # out-DMA per half-chunk on the ACT HWDGE queue
                        bs = slice(cs + (bt - 1) * 128, cs + (bt + 1) * 128)
                        nc.scalar.dma_start(
                            out=t["out"][:, :, bs].rearrange("t o b -> o t b"),
                            in_=outT[:, :, bs],
                        )

                def emit_combine_elem(bt, eo, wn, tw):
                    # towers[b, t, o] = sum_e wn[b, e, t] * eo[b, e, o] as a
                    # broadcast multiply + X-axis reduce, all SBUF->SBUF so
                    # Pool/DVE carry it and the PE never sees it
                    pr = pr_p.tile([128, T, H2, E], F16)
                    wb = wn[:, bt * E * T : (bt + 1) * E * T]
                    # free-dim (X) reduce is DVE-only; gpsimd handles only
                    # partition reductions — so products go to Pool and all
                    # reduces to DVE
                    eng = nc.gpsimd
                    eng2 = nc.vector
                    eng.tensor_mul(
                        pr[:],
                        eo[:]
                        .rearrange("p e o -> p o e")
                        .unsqueeze(1)
                        .broadcast_to([128, T, H2, E]),
                        wb.rearrange("p (e t) -> p t e", t=T)
                        .unsqueeze(2)
                        .broadcast_to([128, T, H2, E]),
                    )
                    # fp16 sum of 8 gate-weighted terms: ~3*2^-11 relative,
                    # far inside the 2e-2 budget
                    with nc.allow_low_precision(reason="8-term fp16 combine"):
                        eng2.reduce_sum(
                            tw[:, bt], pr[:], axis=mybir.AxisListType.X
                        )

                if COMBINE == "elem":
                    wn_c = dgs  # gates_and_diag returned wn in elem mode
                    if "combine" in P:
                        tw = tw_p.tile([128, NBT, T, H2], F16)
                    else:
                        tw = tw_dummy
                    for bt in range(NBT):
                        eo = emit_l2(bt) if "l2" in P else eo_dummy
                        if "combine" in P:
                            emit_combine_elem(bt, eo, wn_c, tw)
                    # chunk out DMA on the (idle) SP queue, one per tower so
                    # the AP balancer sees 3-dim patterns on both sides
                    for t1 in range(T):
                        nc.sync.dma_start(
                            out=t["out"][t1, cs : cs + CHUNK, :].rearrange(
                                "(bt p) o -> p bt o", p=128
                            ),
                            in_=tw[:, :, t1, :],
                        )
                elif "l2" in P and "combine" in P:
                    eos = {}
                    for bt in range(NBT):
                        eos[bt] = emit_l2(bt)
                        if bt >= 1:
                            emit_combine(bt - 1, eos.pop(bt - 1))
                    emit_combine(NBT - 1, eos.pop(NBT - 1))
                elif "l2" in P:
                    for bt in range(NBT):
                        emit_l2(bt)
                elif "combine" in P:
                    for bt in range(NBT):
                        emit_combine(bt, eo_dummy)

        rep_ctx = tc.For_i(0, reps, 1) if reps > 1 else None
        if rep_ctx is not None:
            ctx.enter_context(rep_ctx)
        for _ in range(unroll):
            body()


def _build(has_b2: bool, reps: int = 1, parts: str = "full"):
    nc = bacc.Bacc("TRN2", target_bir_lowering=False, debug=False)
    t = {
        "xt": nc.dram_tensor("xt", [I, BC], F16, kind="ExternalInput").ap(),
        "w1": nc.dram_tensor("w1", [E, I, H1], F16, kind="ExternalInput").ap(),
        "w2": nc.dram_tensor("w2", [E, H1, H2], F16, kind="ExternalInput").ap(),
        "wg": nc.dram_tensor("wg", [I, E * T], F16, kind="ExternalInput").ap(),
        "consts": nc.dram_tensor("consts", [128, 97], F32, kind="ExternalInput").ap(),
        "out": nc.dram_tensor(
            "out",
            [T, H2, BC] if COMBINE == "diag" else [T, BC, H2],
            F16,
            kind="ExternalOutput",
        ).ap(),
    }
    if has_b2:
        t["b2r"] = nc.dram_tensor("b2r", [1, E * H2], F16, kind="ExternalInput").ap()
        t["ones1"] = nc.dram_tensor("ones1", [1, 128], F16, kind="ExternalInput").ap()
    _emit(nc, t, has_b2, reps=reps, parts=parts)
    nc.compile()
    return nc


def _get_nc(has_b2: bool):
    key = ("nc", has_b2)
    if key not in _CACHE:
        _CACHE[key] = _build(has_b2)
    return _CACHE[key]


def _host_consts(b1=None, bg=None):
    """Packed consts: [b1 (16) | ident-f16-as-f32 (64) | id32 (16) | bg (1)]."""
    coh = np.zeros((128, 97), np.float32)
    if b1 is not None:
        coh[:, 0:16] = np.broadcast_to(
            b1.reshape(E, MC_H1, 128).transpose(2, 0, 1), (128, E, MC_H1)
        ).reshape(128, E * MC_H1)
    coh[:, 16:80] = np.ascontiguousarray(np.eye(128, dtype=np.float16)).view(
        np.float32
    )
    coh[:16, 80:96] = np.eye(16, dtype=np.float32)
    if bg is not None:
        coh[:16, 96] = bg.T.reshape(E * T)
    return coh


def _host_prep(x, W1, b1, W2, b2, Wg, bg, has_b2):
    w1h = np.ascontiguousarray(W1.astype(np.float16))
    w2h = np.ascontiguousarray(W2.astype(np.float16))
    wgh = np.ascontiguousarray(
        Wg.transpose(1, 2, 0).reshape(I, E * T).astype(np.float16)
    )
    coh = _host_consts(b1, bg)
    xth = np.ascontiguousarray(x.T.astype(np.float16))
    in_maps = []
    for core in range(NCORES):
        m = {
            "xt": xth[:, core * BC : (core + 1) * BC],
            "w1": w1h,
            "w2": w2h,
            "wg": wgh,
            "consts": coh,
        }
        if has_b2:
            m["b2r"] = b2.astype(np.float16).reshape(1, E * H2)
            m["ones1"] = np.ones((1, 128), np.float16)
        in_maps.append(m)
    return in_maps


def _build_runner(nc):
    """Cached replica of bass2jax.run_bass_via_pjrt's multi-core path: the
    jitted shard_map callable is built once and reused across kernel() calls."""
    import jax
    from jax.experimental.shard_map import shard_map
    from jax.sharding import Mesh, PartitionSpec

    from concourse import bass2jax, mybir as mb

    bass2jax.install_neuronx_cc_hook()
    partition_name = (
        nc.partition_id_tensor.name if nc.partition_id_tensor else None
    )
    in_names, out_names, out_avals, zero_shapes = [], [], [], []
    for alloc in nc.m.functions[0].allocations:
        if not isinstance(mb.MemoryLocationSet, type) or not isinstance(
            alloc, mb.MemoryLocationSet
        ):
            continue
        name = alloc.memorylocations[0].name
        if alloc.kind == "ExternalInput":
            if name != partition_name:
                in_names.append(name)
        elif alloc.kind == "ExternalOutput":
            shape = tuple(alloc.tensor_shape)
            dtype = mb.dt.np(alloc.dtype)
            out_names.append(name)
            out_avals.append(jax.core.ShapedArray(shape, dtype))
            zero_shapes.append((shape, dtype))
    n_params = len(in_names)
    n_outs = len(out_avals)
    all_in_names = list(in_names) + list(out_names)
    if partition_name is not None:
        all_in_names.append(partition_name)
    donate = tuple(range(n_params, n_params + n_outs))

    def _body(*args):
        operands = list(args)
        if partition_name is not None:
            operands.append(bass2jax.partition_id_tensor())
        outs = bass2jax._bass_exec_p.bind(
            *operands,
            out_avals=tuple(out_avals),
            in_names=tuple(all_in_names),
            out_names=tuple(out_names),
            lowering_input_output_aliases=(),
            sim_require_finite=True,
            sim_require_nnan=True,
            nc=nc,
        )
        return tuple(outs)

    devices = jax.devices()[:NCORES]
    mesh = Mesh(np.asarray(devices), ("core",))
    in_specs = (PartitionSpec("core"),) * (n_params + n_outs)
    out_specs = (PartitionSpec("core"),) * n_outs
    sharded = jax.jit(
        shard_map(
            _body, mesh=mesh, in_specs=in_specs, out_specs=out_specs,
            check_rep=False,
        ),
        donate_argnums=donate,
        keep_unused=True,
    )

    def run(in_maps):
        concat_in = [
            np.concatenate([np.asarray(m[name]) for m in in_maps], axis=0)
            for name in in_names
        ]
        concat_zeros = [
            np.zeros((NCORES * s[0], *s[1:]), d) for s, d in zero_shapes
        ]
        out_arrs = sharded(*concat_in, *concat_zeros)
        return [
            {
                name: np.asarray(out_arrs[i]).reshape(
                    NCORES, *zero_shapes[i][0]
                )[c]
                for i, name in enumerate(out_names)
            }
            for c in range(NCORES)
        ]

    return run


def kernel(x, W1, b1, W2, b2, Wg, bg):
    x = np.asarray(x, np.float32)
    W1 = np.asarray(W1, np.float32)
    b1 = np.asarray(b1, np.float32)
    W2 = np.asarray(W2, np.float32)
    b2 = np.asarray(b2, np.float32)
    Wg = np.asarray(Wg, np.float32)
    bg = np.asarray(bg, np.float32)

    has_b2 = bool(np.any(b2))
    nc = _get_nc(has_b2)
    in_maps = _host_prep(x, W1, b1, W2, b2, Wg, bg, has_b2)

    key = ("runner", has_b2)
    try:
        if key not in _CACHE:
            _CACHE[key] = _build_runner(nc)
        results = _CACHE[key](in_maps)
    except Exception:
        _CACHE.pop(key, None)
        results = run_bass_kernel_spmd(
            nc, in_maps, core_ids=list(range(NCORES))
        ).results
    # gather: per-core out fp16 -> [T, B, H2] fp32
    if COMBINE == "diag":
        outs = [r["out"].transpose(0, 2, 1) for r in results]
    else:
        outs = [r["out"] for r in results]
    return np.ascontiguousarray(np.concatenate(outs, axis=1), dtype=np.float32)
